# revision 12
# baseline (speedup 1.0000x reference)
"""Distributed Trainium2 kernel for ANEMultiHeadAttention.

Problem: B=2, C=1024, S=2048, H=16, D=64.
  x: (B, C, 1, S);  q = Wq x + bq; k = Wk x; v = Wv x + bv
  per-head attention (softmax over keys), out = Wo o + bo.

Sharding (8 cores): core i handles batch b = i // 4 and head-group
hg = i % 4 (4 heads = 256 channels). Q/K/V column-parallel, Wo
row-parallel; host sums the 4 partial outputs per batch.
The v-bias contributes Wo @ bv (softmax rows sum to 1) and is folded
into a host-side constant along with bo.

Per-core device algorithm (all matmuls bf16, f32 PSUM):
  - q = WqT_s^T @ x (+bq), k = WkT_s^T @ x, computed in (pair, 1024)
    units; vT = x^T @ WvT_s stored (128, head, 65) per key-tile with a
    ones column (PV then also accumulates softmax denominators).
  - attention per q-window of 1024, per head pair: scoresT = k^T q
    (row-packed pair via tile_position), exp on ACT (scale 1/8),
    PV: o_aug += vT_aug^T @ expT.  The exp stream on the Scalar engine
    is the critical path; QKV units are emission-interleaved into the
    first window so the PE stays dense (and HAM-warm) while ACT works.
  - normalize (recip + partition_broadcast + mul) off the critical
    path after a fast PSUM->SBUF evac; out-projection per window is
    deferred one attention block so it never stalls the exp stream.
"""

import sys

for p in ("/opt/trn_rl_repo",):
    if p not in sys.path:
        sys.path.insert(0, p)

from contextlib import ExitStack

import ml_dtypes
import numpy as np

import concourse.bass as bass
import concourse.mybir as mybir
import concourse.tile as tile
from concourse import bacc
from concourse.bass_utils import run_bass_kernel_spmd

# Problem shape (hardcoded per contest rules)
B, C, S, H = 2, 1024, 2048, 16
D = C // H  # 64
N_CORES = 8
HG = 4  # head groups
HPG = H // HG  # heads per group = 4
CPG = HPG * D  # channels per group = 256
P = 128
NK = C // P  # 8 contraction tiles over C
NST = S // P  # 16 key tiles
WIN = 1024  # q window
NWIN = S // WIN  # 2
NCH = WIN // 512  # 512-chunks per window = 2

F32 = mybir.dt.float32
BF16 = mybir.dt.bfloat16
EXP = mybir.ActivationFunctionType.Exp

_CACHED_NC = None


def build_nc():
    nc = bacc.Bacc("TRN2", target_bir_lowering=False, debug=False)

    x_d = nc.dram_tensor("x", (P, NK, S), BF16, kind="ExternalInput")
    wq_d = nc.dram_tensor("wqT", (P, NK, CPG), BF16, kind="ExternalInput")
    wk_d = nc.dram_tensor("wkT", (P, NK, CPG), BF16, kind="ExternalInput")
    wv_d = nc.dram_tensor("wvT", (P, NK, CPG), BF16, kind="ExternalInput")
    wo_d = nc.dram_tensor("woT", (P, 2, C), BF16, kind="ExternalInput")
    bq_d = nc.dram_tensor("bq", (P, 2), F32, kind="ExternalInput")
    out_d = nc.dram_tensor("out", (P, NK, S), F32, kind="ExternalOutput")

    with tile.TileContext(nc) as tc, ExitStack() as ctx:
        const = ctx.enter_context(tc.tile_pool(name="const", bufs=1))
        work = ctx.enter_context(tc.tile_pool(name="work", bufs=1))
        expp = ctx.enter_context(tc.tile_pool(name="expp", bufs=8))
        onp = ctx.enter_context(tc.tile_pool(name="onp", bufs=4))
        outp = ctx.enter_context(tc.tile_pool(name="outp", bufs=3))
        smal = ctx.enter_context(tc.tile_pool(name="smal", bufs=4))
        psc = ctx.enter_context(tc.tile_pool(name="psc", bufs=2, space="PSUM"))
        ppv = ctx.enter_context(tc.tile_pool(name="ppv", bufs=2, space="PSUM"))

        # ---- inputs in SBUF; x split per k-tile so DMA pipelines ----
        xt = []
        for kt in range(NK):
            t = const.tile([P, S], BF16, tag=f"x{kt}", name=f"x{kt}")
            nc.sync.dma_start(t[:], x_d[:, kt, :])
            xt.append(t)
        wq_sb = const.tile([P, NK, CPG], BF16, tag="wq")
        nc.sync.dma_start(wq_sb[:], wq_d[:])
        wk_sb = const.tile([P, NK, CPG], BF16, tag="wk")
        nc.sync.dma_start(wk_sb[:], wk_d[:])
        wv_sb = const.tile([P, NK, CPG], BF16, tag="wv")
        nc.sync.dma_start(wv_sb[:], wv_d[:])
        wo_sb = const.tile([P, 2, C], BF16, tag="wo")
        nc.sync.dma_start(wo_sb[:], wo_d[:])
        bq_sb = const.tile([P, 2], F32, tag="bq")
        nc.sync.dma_start(bq_sb[:], bq_d[:])

        # activations, one tile per (pair, window/chunk) for fine deps
        def wtile(nm):
            return work.tile([P, WIN], BF16, tag=nm, name=nm)

        k_t = [[wtile(f"k{p_}c{c}") for c in range(2)] for p_ in range(2)]
        q_t = [[wtile(f"q{p_}w{w}") for w in range(2)] for p_ in range(2)]
        o_t = [[wtile(f"o{p_}w{w}") for w in range(2)] for p_ in range(2)]
        vt = [
            work.tile([P, HPG, D + 1], BF16, tag=f"vt{st}", name=f"vt{st}")
            for st in range(NST)
        ]

        def qk_mms(ps, w_sb, pair, c, kt):
            for ch in range(NCH):
                nc.tensor.matmul(
                    ps[:, ch * 512 : (ch + 1) * 512],
                    w_sb[:, kt, pair * P : (pair + 1) * P],
                    xt[kt][:, c * WIN + ch * 512 : c * WIN + (ch + 1) * 512],
                    start=(kt == 0),
                    stop=(kt == NK - 1),
                )

        def qk_evac(ps, dst, pair, bias):
            if bias:
                nc.vector.tensor_scalar_add(
                    dst[:], ps[:], bq_sb[:, pair : pair + 1]
                )
            else:
                nc.vector.tensor_copy(dst[:], ps[:])

        def qk_unit(w_sb, dst, pair, c, bias):
            ps = psc.tile([P, WIN], F32, tag="psc", name="ps_qk")
            for kt in range(NK):
                qk_mms(ps, w_sb, pair, c, kt)
            qk_evac(ps, dst, pair, bias)

        def vt_mm(ps, st, kt):
            nc.tensor.matmul(
                ps[:, :CPG],
                xt[kt][:, st * P : (st + 1) * P],
                wv_sb[:, kt, :],
                start=(kt == 0),
                stop=(kt == NK - 1),
            )

        def vt_evac(ps, st):
            nc.vector.tensor_copy(
                vt[st][:, :, 0:D],
                ps[:, :CPG].rearrange("p (h d) -> p h d", h=HPG),
            )

        def vt_unit(st):
            nc.vector.memset(vt[st][:], 1.0)
            ps = psc.tile([P, WIN], F32, tag="psc", name="ps_vt")
            for kt in range(NK):
                vt_mm(ps, st, kt)
            vt_evac(ps, st)

        def attention(w, pair, inject=None):
            inject = inject or {}
            oa = ppv.tile([P, WIN], F32, tag="ppv", name="oa")
            ob = ppv.tile([P, WIN], F32, tag="ppv", name="ob")

            def pv(prev):
                # head-major: consecutive chunk MMs share the stationary
                # operand (no LDWEIGHTS between them -> pipelined N-cycle gap)
                pkt, pea, peb = prev
                for acc, e_, hoff in ((oa, pea, 0), (ob, peb, 1)):
                    for ch in range(NCH):
                        cs = slice(ch * 512, (ch + 1) * 512)
                        nc.tensor.matmul(
                            acc[0 : D + 1, cs],
                            vt[pkt][:, 2 * pair + hoff, :],
                            e_[:, cs],
                            start=(pkt == 0),
                            stop=(pkt == NST - 1),
                        )

            prev = None
            for kt in range(NST):
                sa = psc.tile([P, WIN], F32, tag="psc", name="sa")
                sb = psc.tile([P, WIN], F32, tag="psc", name="sb")
                c, j = divmod(kt, NK)
                for dst, rlo, rhi, tpos in (
                    (sa, 0, D, (0, 0)),
                    (sb, D, P, (64, 0)),
                ):
                    for ch in range(NCH):
                        cs = slice(ch * 512, (ch + 1) * 512)
                        nc.tensor.matmul(
                            dst[:, cs],
                            k_t[pair][c][rlo:rhi, j * P : (j + 1) * P],
                            q_t[pair][w][rlo:rhi, cs],
                            tile_position=tpos,
                        )
                ea = expp.tile([P, WIN], BF16, tag="exp", name="ea")
                eb = expp.tile([P, WIN], BF16, tag="exp", name="eb")
                nc.scalar.activation(ea[:], sa[:], EXP, scale=float(D) ** -0.5)
                nc.scalar.activation(eb[:], sb[:], EXP, scale=float(D) ** -0.5)
                if prev is not None:
                    pv(prev)
                prev = (kt, ea, eb)
                for f in inject.get(kt, ()):
                    f()
            pv(prev)

            # fast evac to SBUF first (frees both PSUM slots before the slow
            # reciprocal runs), then normalize off the hot path
            ous = []
            for acc in (oa, ob):
                ou = onp.tile([D + 1, WIN], F32, tag="ou", name="ou")
                nc.vector.tensor_copy(ou[:], acc[0 : D + 1, :])
                ous.append(ou)
            for head, ou in enumerate(ous):
                rc = smal.tile([1, WIN], F32, tag="rc", name="rc")
                nc.vector.reciprocal(rc[:], ou[D : D + 1, :])
                rcb = smal.tile([D, WIN], F32, tag="rcb", name="rcb")
                nc.gpsimd.partition_broadcast(rcb[:], rc[:])
                nc.vector.tensor_mul(
                    o_t[pair][w][head * D : (head + 1) * D, :], ou[0:D, :], rcb[:]
                )

        def outproj_unit(w, m, pool):
            ps = pool.tile([P, WIN], F32, tag=pool is psc and "psc" or "ppv", name="ps_out")
            for kt in range(2):
                for ch in range(NCH):
                    cs = slice(ch * 512, (ch + 1) * 512)
                    nc.tensor.matmul(
                        ps[:, cs],
                        wo_sb[:, kt, m * P : (m + 1) * P],
                        o_t[kt][w][:, cs],
                        start=(kt == 0),
                        stop=(kt == 1),
                    )
            ot = outp.tile([P, WIN], F32, tag="ot", name="ot")
            nc.vector.tensor_copy(ot[:], ps[:])
            nc.sync.dma_start(out_d[:, m, w * WIN : (w + 1) * WIN], ot[:])

        # ---- emission schedule ----
        # Head phase: k/q for pair 0 window 0 plus the first two vT key
        # tiles, interleaved per contraction tile so the x DMAs pipeline.
        ps_k = psc.tile([P, WIN], F32, tag="psc", name="ps_k")
        ps_q = psc.tile([P, WIN], F32, tag="psc", name="ps_q")
        ps_v0 = ppv.tile([P, WIN], F32, tag="ppv", name="ps_v0")
        ps_v1 = ppv.tile([P, WIN], F32, tag="ppv", name="ps_v1")
        nc.vector.memset(vt[0][:], 1.0)
        nc.vector.memset(vt[1][:], 1.0)
        for kt in range(NK):
            qk_mms(ps_k, wk_sb, 0, 0, kt)
            qk_mms(ps_q, wq_sb, 0, 0, kt)
            vt_mm(ps_v0, 0, kt)
            vt_mm(ps_v1, 1, kt)
        qk_evac(ps_k, k_t[0][0], 0, False)
        qk_evac(ps_q, q_t[0][0], 0, True)
        vt_evac(ps_v0, 0)
        vt_evac(ps_v1, 1)

        def U(f, *a):
            return lambda: f(*a)

        inj00 = {
            0: (U(vt_unit, 2), U(vt_unit, 3)),
            1: (U(vt_unit, 4), U(vt_unit, 5)),
            2: (U(vt_unit, 6), U(vt_unit, 7)),
            3: (U(qk_unit, wk_sb, k_t[0][1], 0, 1, False),),
            4: (U(vt_unit, 8), U(vt_unit, 9)),
            5: (U(vt_unit, 10), U(vt_unit, 11)),
            6: (U(vt_unit, 12), U(vt_unit, 13)),
            7: (U(vt_unit, 14), U(vt_unit, 15)),
            8: (U(qk_unit, wk_sb, k_t[1][0], 1, 0, False),),
            10: (U(qk_unit, wq_sb, q_t[1][0], 1, 0, True),),
            12: (U(qk_unit, wk_sb, k_t[1][1], 1, 1, False),),
            14: (U(qk_unit, wq_sb, q_t[0][1], 0, 1, True),),
        }
        attention(0, 0, inj00)
        inj01 = {2: (U(qk_unit, wq_sb, q_t[1][1], 1, 1, True),)}
        attention(0, 1, inj01)
        attention(1, 0)
        inj11 = {
            2 * m + 1: (U(outproj_unit, 0, m, psc),) for m in range(NK)
        }
        attention(1, 1, inj11)
        for m in range(NK):
            outproj_unit(1, m, ppv)

    nc.compile()
    return nc


def _shard_inputs(hidden_states, Wq, bq, Wk, Wv, bv, Wo, bo):
    bf = ml_dtypes.bfloat16
    in_maps = []
    for core in range(N_CORES):
        b, hg = divmod(core, HG)
        x = hidden_states[b, :, 0, :]  # (C, S) f32
        cs = slice(hg * CPG, (hg + 1) * CPG)
        wqT = Wq[cs, :].T.reshape(NK, P, CPG).transpose(1, 0, 2)
        wkT = Wk[cs, :].T.reshape(NK, P, CPG).transpose(1, 0, 2)
        wvT = Wv[cs, :].T.reshape(NK, P, CPG).transpose(1, 0, 2)
        woT = Wo[:, cs].T.reshape(2, P, C).transpose(1, 0, 2)
        in_maps.append(
            {
                "x": np.ascontiguousarray(
                    x.reshape(NK, P, S).transpose(1, 0, 2)
                ).astype(bf),
                "wqT": np.ascontiguousarray(wqT).astype(bf),
                "wkT": np.ascontiguousarray(wkT).astype(bf),
                "wvT": np.ascontiguousarray(wvT).astype(bf),
                "woT": np.ascontiguousarray(woT).astype(bf),
                "bq": np.ascontiguousarray(
                    bq[cs].reshape(2, P).T
                ).astype(np.float32),
            }
        )
    return in_maps


def get_nc():
    global _CACHED_NC
    if _CACHED_NC is None:
        _CACHED_NC = build_nc()
    return _CACHED_NC


def run(hidden_states, Wq, bq, Wk, Wv, bv, Wo, bo, trace=False, **kw):
    nc = get_nc()
    in_maps = _shard_inputs(hidden_states, Wq, bq, Wk, Wv, bv, Wo, bo)
    res = run_bass_kernel_spmd(
        nc, in_maps, core_ids=list(range(N_CORES)), trace=trace, **kw
    )
    # unshard: sum partials per batch, add host-side constant bias
    bias_vec = (Wo.astype(np.float64) @ bv.astype(np.float64)).astype(
        np.float32
    ) + bo
    out = np.zeros((B, C, 1, S), dtype=np.float32)
    for core in range(N_CORES):
        b = core // HG
        part = np.asarray(res.results[core]["out"], dtype=np.float32)
        out[b, :, 0, :] += part.transpose(1, 0, 2).reshape(C, S)
    out[:, :, 0, :] += bias_vec[None, :, None]
    return out, res


def kernel(**inputs):
    out, _ = run(**inputs)
    return out


# revision 15
# speedup vs baseline: 1.4611x; 1.4611x over previous
"""Distributed Trainium2 kernel for ANEMultiHeadAttention.

Problem: B=2, C=1024, S=2048, H=16, D=64.
  x: (B, C, 1, S);  q = Wq x + bq; k = Wk x; v = Wv x + bv
  per-head attention (softmax over keys), out = Wo o + bo.

Sharding (8 cores): core i handles batch b = i // 4 and head-group
hg = i % 4 (4 heads = 256 channels). Q/K/V column-parallel, Wo
row-parallel; host sums the 4 partial outputs per batch.
The v-bias contributes Wo @ bv (softmax rows sum to 1) and is folded
into a host-side constant along with bo.

Per-core device algorithm (all matmuls bf16, f32 PSUM):
  - q = WqT_s^T @ x (+bq), k = WkT_s^T @ x; vT = x^T @ WvT_s stored
    (128, head, 65) per key-tile with a ones column (the PV matmul then
    also accumulates softmax denominators).
  - attention in q-windows of 512, head pairs row-packed: BOTH heads'
    scoresT land in ONE (128, 1024) PSUM tile (different banks), one
    exp instruction (ACT, scale 1/8) covers both heads -> the exp
    stream on the Scalar engine runs back-to-back (it is the kernel's
    critical path); PV: o_aug += vT_aug^T @ expT per head.
  - QKV/vT/out-proj work is emission-interleaved into the attention
    loops so the PE stays dense while ACT works.
  - normalize (recip + partition_broadcast + mul) runs off the hot
    path after a fast PSUM->SBUF evac.
"""

import sys

for p in ("/opt/trn_rl_repo",):
    if p not in sys.path:
        sys.path.insert(0, p)

from contextlib import ExitStack

import ml_dtypes
import numpy as np

import concourse.bass as bass
import concourse.mybir as mybir
import concourse.tile as tile
from concourse import bacc
from concourse.bass_utils import run_bass_kernel_spmd

# Problem shape (hardcoded per contest rules)
B, C, S, H = 2, 1024, 2048, 16
D = C // H  # 64
N_CORES = 8
HG = 4  # head groups
HPG = H // HG  # heads per group = 4
CPG = HPG * D  # channels per group = 256
P = 128
NK = C // P  # 8 contraction tiles over C
NST = S // P  # 16 key tiles
QW = 512  # q window per head
NQW = S // QW  # 4
WIN = 1024  # qk-projection unit width

F32 = mybir.dt.float32
BF16 = mybir.dt.bfloat16
EXP = mybir.ActivationFunctionType.Exp

_CACHED_NC = None


def build_nc():
    nc = bacc.Bacc("TRN2", target_bir_lowering=False, debug=False)

    x_d = nc.dram_tensor("x", (P, NK, S), BF16, kind="ExternalInput")
    wq_d = nc.dram_tensor("wqT", (P, NK, CPG), BF16, kind="ExternalInput")
    wk_d = nc.dram_tensor("wkT", (P, NK, CPG), BF16, kind="ExternalInput")
    wv_d = nc.dram_tensor("wvT", (P, NK, CPG), BF16, kind="ExternalInput")
    wo_d = nc.dram_tensor("woT", (P, 2, C), BF16, kind="ExternalInput")
    bq_d = nc.dram_tensor("bq", (P, 2), F32, kind="ExternalInput")
    out_d = nc.dram_tensor("out", (P, NK, S), F32, kind="ExternalOutput")

    with tile.TileContext(nc) as tc, ExitStack() as ctx:
        const = ctx.enter_context(tc.tile_pool(name="const", bufs=1))
        work = ctx.enter_context(tc.tile_pool(name="work", bufs=1))
        expp = ctx.enter_context(tc.tile_pool(name="expp", bufs=8))
        onp = ctx.enter_context(tc.tile_pool(name="onp", bufs=6))
        outp = ctx.enter_context(tc.tile_pool(name="outp", bufs=3))
        smal = ctx.enter_context(tc.tile_pool(name="smal", bufs=6))
        psc = ctx.enter_context(tc.tile_pool(name="psc", bufs=2, space="PSUM"))
        ppv = ctx.enter_context(tc.tile_pool(name="ppv", bufs=4, space="PSUM"))

        # ---- inputs in SBUF; x split per k-tile so DMA pipelines ----
        xt = []
        for kt in range(NK):
            t = const.tile([P, S], BF16, tag=f"x{kt}", name=f"x{kt}")
            nc.sync.dma_start(t[:], x_d[:, kt, :])
            xt.append(t)
        wq_sb = const.tile([P, NK, CPG], BF16, tag="wq")
        nc.sync.dma_start(wq_sb[:], wq_d[:])
        wk_sb = const.tile([P, NK, CPG], BF16, tag="wk")
        nc.sync.dma_start(wk_sb[:], wk_d[:])
        wv_sb = const.tile([P, NK, CPG], BF16, tag="wv")
        nc.sync.dma_start(wv_sb[:], wv_d[:])
        wo_sb = const.tile([P, 2, C], BF16, tag="wo")
        nc.sync.dma_start(wo_sb[:], wo_d[:])
        bq_sb = const.tile([P, 2], F32, tag="bq")
        nc.sync.dma_start(bq_sb[:], bq_d[:])

        # activations, one tile per (pair, window/chunk) for fine deps
        def wtile(nm):
            return work.tile([P, WIN], BF16, tag=nm, name=nm)

        k_t = [[wtile(f"k{p_}c{c}") for c in range(2)] for p_ in range(2)]
        q_t = [[wtile(f"q{p_}w{w}") for w in range(2)] for p_ in range(2)]
        o_t = [[wtile(f"o{p_}w{w}") for w in range(2)] for p_ in range(2)]
        vt = [
            work.tile([P, HPG, D + 1], BF16, tag=f"vt{st}", name=f"vt{st}")
            for st in range(NST)
        ]

        def qk_mms(ps, w_sb, pair, c, kt):
            for ch in range(2):
                nc.tensor.matmul(
                    ps[:, ch * 512 : (ch + 1) * 512],
                    w_sb[:, kt, pair * P : (pair + 1) * P],
                    xt[kt][:, c * WIN + ch * 512 : c * WIN + (ch + 1) * 512],
                    start=(kt == 0),
                    stop=(kt == NK - 1),
                )

        def qk_evac(ps, dst, pair, bias):
            if bias:
                nc.vector.tensor_scalar_add(
                    dst[:], ps[:], bq_sb[:, pair : pair + 1]
                )
            else:
                nc.vector.tensor_copy(dst[:], ps[:])

        def qk_unit(w_sb, dst, pair, c, bias):
            ps = psc.tile([P, WIN], F32, tag="psc", name="ps_qk")
            for kt in range(NK):
                qk_mms(ps, w_sb, pair, c, kt)
            qk_evac(ps, dst, pair, bias)

        def vt_mm(ps, st, kt):
            nc.tensor.matmul(
                ps[:, :CPG],
                xt[kt][:, st * P : (st + 1) * P],
                wv_sb[:, kt, :],
                start=(kt == 0),
                stop=(kt == NK - 1),
            )

        def vt_evac(ps, st):
            nc.vector.tensor_copy(
                vt[st][:, :, 0:D],
                ps[:, :CPG].rearrange("p (h d) -> p h d", h=HPG),
            )

        def vt_unit(st):
            nc.vector.memset(vt[st][:], 1.0)
            ps = ppv.tile([P, QW], F32, tag="ppv", name="ps_vt")
            for kt in range(NK):
                vt_mm(ps, st, kt)
            vt_evac(ps, st)

        def attention(pair, qw, inject=None):
            inject = inject or {}
            w, half = divmod(qw, 2)
            qs = slice(half * QW, (half + 1) * QW)
            oa = ppv.tile([P, QW], F32, tag="ppv", name="oa")
            ob = ppv.tile([P, QW], F32, tag="ppv", name="ob")

            def pv(prev):
                pkt, pe = prev
                for acc, hoff, cs in (
                    (oa, 0, slice(0, QW)),
                    (ob, 1, slice(QW, 2 * QW)),
                ):
                    nc.tensor.matmul(
                        acc[0 : D + 1, :],
                        vt[pkt][:, 2 * pair + hoff, :],
                        pe[:, cs],
                        start=(pkt == 0),
                        stop=(pkt == NST - 1),
                    )

            prev = None
            for kt in range(NST):
                s = psc.tile([P, WIN], F32, tag="psc", name="s")
                c, j = divmod(kt, NK)
                for rlo, rhi, cs, tpos in (
                    (0, D, slice(0, QW), (0, 0)),
                    (D, P, slice(QW, 2 * QW), (64, 0)),
                ):
                    nc.tensor.matmul(
                        s[:, cs],
                        k_t[pair][c][rlo:rhi, j * P : (j + 1) * P],
                        q_t[pair][w][rlo:rhi, qs],
                        tile_position=tpos,
                    )
                e = expp.tile([P, WIN], BF16, tag="exp", name="e")
                nc.scalar.activation(e[:], s[:], EXP, scale=float(D) ** -0.5)
                if prev is not None:
                    pv(prev)
                prev = (kt, e)
                for f in inject.get(kt, ()):
                    f()
            pv(prev)

            # fast evac to SBUF first (frees both PSUM slots before the slow
            # reciprocal runs), then normalize off the hot path
            ous = []
            for acc in (oa, ob):
                ou = onp.tile([D + 1, QW], F32, tag="ou", name="ou")
                nc.vector.tensor_copy(ou[:], acc[0 : D + 1, :])
                ous.append(ou)
            for head, ou in enumerate(ous):
                rc = smal.tile([1, QW], F32, tag="rc", name="rc")
                nc.vector.reciprocal(rc[:], ou[D : D + 1, :])
                rcb = smal.tile([D, QW], F32, tag="rcb", name="rcb")
                nc.gpsimd.partition_broadcast(rcb[:], rc[:])
                nc.vector.tensor_mul(
                    o_t[pair][w][head * D : (head + 1) * D, qs],
                    ou[0:D, :],
                    rcb[:],
                )

        def outproj_unit(w, m):
            ps = psc.tile([P, WIN], F32, tag="psc", name="ps_out")
            for kt in range(2):
                for ch in range(2):
                    cs = slice(ch * 512, (ch + 1) * 512)
                    nc.tensor.matmul(
                        ps[:, cs],
                        wo_sb[:, kt, m * P : (m + 1) * P],
                        o_t[kt][w][:, cs],
                        start=(kt == 0),
                        stop=(kt == 1),
                    )
            ot = outp.tile([P, WIN], F32, tag="ot", name="ot")
            nc.vector.tensor_copy(ot[:], ps[:])
            nc.sync.dma_start(out_d[:, m, w * WIN : (w + 1) * WIN], ot[:])

        # ---- emission schedule ----
        # Head phase: k/q for pair 0 window 0 plus the first two vT key
        # tiles, interleaved per contraction tile so the x DMAs pipeline.
        ps_k = psc.tile([P, WIN], F32, tag="psc", name="ps_k")
        ps_q = psc.tile([P, WIN], F32, tag="psc", name="ps_q")
        ps_v0 = ppv.tile([P, QW], F32, tag="ppv", name="ps_v0")
        ps_v1 = ppv.tile([P, QW], F32, tag="ppv", name="ps_v1")
        nc.vector.memset(vt[0][:], 1.0)
        nc.vector.memset(vt[1][:], 1.0)
        for kt in range(NK):
            qk_mms(ps_k, wk_sb, 0, 0, kt)
            qk_mms(ps_q, wq_sb, 0, 0, kt)
            vt_mm(ps_v0, 0, kt)
            vt_mm(ps_v1, 1, kt)
        qk_evac(ps_k, k_t[0][0], 0, False)
        qk_evac(ps_q, q_t[0][0], 0, True)
        vt_evac(ps_v0, 0)
        vt_evac(ps_v1, 1)

        def U(f, *a):
            return lambda: f(*a)

        VT = lambda st: U(vt_unit, st)  # noqa: E731
        QK = lambda wsb, dst, p_, c_, b_: U(qk_unit, wsb, dst, p_, c_, b_)  # noqa: E731

        # attention(0, qw0): vt 2..15 (ppv slots), k p0c1 (psc)
        attention(
            0,
            0,
            {
                0: (VT(2), VT(3)),
                2: (VT(4), VT(5)),
                4: (VT(6), VT(7), QK(wk_sb, k_t[0][1], 0, 1, False)),
                6: (VT(8), VT(9)),
                8: (VT(10), VT(11)),
                10: (VT(12), VT(13)),
                12: (VT(14), VT(15)),
            },
        )
        attention(
            0,
            1,
            {
                2: (QK(wk_sb, k_t[1][0], 1, 0, False),),
                8: (QK(wq_sb, q_t[1][0], 1, 0, True),),
                14: (QK(wk_sb, k_t[1][1], 1, 1, False),),
            },
        )
        attention(1, 0, {})
        attention(1, 1, {4: (QK(wq_sb, q_t[0][1], 0, 1, True),)})
        # out-projection for the first 1024 window, interleaved
        attention(
            0,
            2,
            {
                2: (U(outproj_unit, 0, 0),),
                4: (QK(wq_sb, q_t[1][1], 1, 1, True),),
                6: (U(outproj_unit, 0, 1),),
                10: (U(outproj_unit, 0, 2),),
                14: (U(outproj_unit, 0, 3),),
            },
        )
        attention(
            0,
            3,
            {
                2: (U(outproj_unit, 0, 4),),
                6: (U(outproj_unit, 0, 5),),
                10: (U(outproj_unit, 0, 6),),
                14: (U(outproj_unit, 0, 7),),
            },
        )
        attention(1, 2, {})
        attention(1, 3, {})
        for m in range(NK):
            outproj_unit(1, m)

    nc.compile()
    return nc


def _shard_inputs(hidden_states, Wq, bq, Wk, Wv, bv, Wo, bo):
    bf = ml_dtypes.bfloat16
    in_maps = []
    for core in range(N_CORES):
        b, hg = divmod(core, HG)
        x = hidden_states[b, :, 0, :]  # (C, S) f32
        cs = slice(hg * CPG, (hg + 1) * CPG)
        wqT = Wq[cs, :].T.reshape(NK, P, CPG).transpose(1, 0, 2)
        wkT = Wk[cs, :].T.reshape(NK, P, CPG).transpose(1, 0, 2)
        wvT = Wv[cs, :].T.reshape(NK, P, CPG).transpose(1, 0, 2)
        woT = Wo[:, cs].T.reshape(2, P, C).transpose(1, 0, 2)
        in_maps.append(
            {
                "x": np.ascontiguousarray(
                    x.reshape(NK, P, S).transpose(1, 0, 2)
                ).astype(bf),
                "wqT": np.ascontiguousarray(wqT).astype(bf),
                "wkT": np.ascontiguousarray(wkT).astype(bf),
                "wvT": np.ascontiguousarray(wvT).astype(bf),
                "woT": np.ascontiguousarray(woT).astype(bf),
                "bq": np.ascontiguousarray(
                    bq[cs].reshape(2, P).T
                ).astype(np.float32),
            }
        )
    return in_maps


def get_nc():
    global _CACHED_NC
    if _CACHED_NC is None:
        _CACHED_NC = build_nc()
    return _CACHED_NC


def run(hidden_states, Wq, bq, Wk, Wv, bv, Wo, bo, trace=False, **kw):
    nc = get_nc()
    in_maps = _shard_inputs(hidden_states, Wq, bq, Wk, Wv, bv, Wo, bo)
    res = run_bass_kernel_spmd(
        nc, in_maps, core_ids=list(range(N_CORES)), trace=trace, **kw
    )
    # unshard: sum partials per batch, add host-side constant bias
    bias_vec = (Wo.astype(np.float64) @ bv.astype(np.float64)).astype(
        np.float32
    ) + bo
    out = np.zeros((B, C, 1, S), dtype=np.float32)
    for core in range(N_CORES):
        b = core // HG
        part = np.asarray(res.results[core]["out"], dtype=np.float32)
        out[b, :, 0, :] += part.transpose(1, 0, 2).reshape(C, S)
    out[:, :, 0, :] += bias_vec[None, :, None]
    return out, res


def kernel(**inputs):
    out, _ = run(**inputs)
    return out
